# revision 1
# baseline (speedup 1.0000x reference)
"""Data-parallel kernel for nn_AdaptiveNeighborhoodFusionTransformer.

Strategy (per sharding hint): pure data parallel over batch B=1024 across the
8 NeuronCores -> 128 batch items per core, params replicated. The whole
network is compiled per-shard and executed on all 8 cores via jax.pmap on the
axon/neuron backend; outputs are gathered back to the full [1024,768,2,2].

Self-contained: shapes hardcoded (B=1024, D=768, H=W=2, L=4, NH=8, 8 cores).
"""

import numpy as np
import jax
import jax.numpy as jnp

D = 768
NH = 8
B = 1024
HH = 2
WW = 2
L = HH * WW
N_CORES = 8


def _gelu(x):
    return jax.nn.gelu(x, approximate=False)


def _layernorm(x, g, b, eps=1e-5):
    m = x.mean(-1, keepdims=True)
    v = ((x - m) ** 2).mean(-1, keepdims=True)
    return (x - m) / jnp.sqrt(v + eps) * g + b


def _lin(x, p):
    return x @ p['w'].T + p['b']


def _cross_attn(xa, xb, p):
    Bb, Ll, Dd = xa.shape
    hd = Dd // NH
    scale = hd ** -0.5
    an = _layernorm(xa, p['na_g'], p['na_b'])
    bn = _layernorm(xb, p['nb_g'], p['nb_b'])

    def heads(x, w):
        return (x @ w.T).reshape(Bb, Ll, NH, hd).transpose(0, 2, 1, 3)

    qa, ka, va = heads(an, p['wq_a']), heads(an, p['wk_a']), heads(an, p['wv_a'])
    qb, kb, vb = heads(bn, p['wq_b']), heads(bn, p['wk_b']), heads(bn, p['wv_b'])
    t = jnp.clip(p['temp'], 0.01)
    A_ab = jax.nn.softmax(jnp.einsum('bhqd,bhkd->bhqk', qa, kb) * scale / t, axis=-1)
    A_ba = jax.nn.softmax(jnp.einsum('bhqd,bhkd->bhqk', qb, ka) * scale / t, axis=-1)
    oa = jnp.einsum('bhqk,bhkd->bhqd', A_ab, vb).transpose(0, 2, 1, 3).reshape(Bb, Ll, Dd)
    ob = jnp.einsum('bhqk,bhkd->bhqd', A_ba, va).transpose(0, 2, 1, 3).reshape(Bb, Ll, Dd)
    cross = _lin(_gelu(_lin(jnp.concatenate([oa, ob], -1), p['cf1'])), p['cf2'])
    xe = xa + jnp.tanh(p['alpha']) * cross
    ffn = _lin(_gelu(_lin(xe, p['ffn1'])), p['ffn2'])
    xe = xe + jnp.tanh(p['beta']) * ffn
    return _lin(xe, p['out'])


def _multiscale(x, P):
    Ll = x.shape[1]
    xc = x.transpose(0, 2, 1)  # [B, D, L]
    outs = []
    for cp in P['ms_convs']:
        k = cp['w'].shape[-1]
        pad = k // 2
        y = jax.lax.conv_general_dilated(xc, cp['w'], (1,), [(pad, pad)],
                                         dimension_numbers=('NCH', 'OIH', 'NCH'),
                                         feature_group_count=D // 8)
        y = (y + cp['b'][None, :, None])[..., :Ll]
        y = y / jnp.sqrt(1.0 + 1e-5) * cp['bn_g'][None, :, None] + cp['bn_b'][None, :, None]
        outs.append(_gelu(y).transpose(0, 2, 1))
    ms = jnp.concatenate(outs, -1)  # [B, L, 3D]
    fused = _lin(_gelu(_lin(ms, P['ms_f1'])), P['ms_f2'])
    return fused + x


def _spatial_relation(feats, P):
    out = []
    for i, f in enumerate(feats):
        sw = jax.nn.sigmoid(_lin(jax.nn.relu(_lin(f, P['sp1'])), P['sp2']))
        out.append(f + P['pos_embed'][i][None] * sw)
    return out


def _mha(x, P):
    Bb, Ll, Dd = x.shape
    hd = Dd // NH
    qkv = x @ P['attn_in_w'].T + P['attn_in_b']
    q, k, v = jnp.split(qkv, 3, -1)

    def h(t):
        return t.reshape(Bb, Ll, NH, hd).transpose(0, 2, 1, 3)

    q, k, v = h(q), h(k), h(v)
    A = jax.nn.softmax(jnp.einsum('bhqd,bhkd->bhqk', q, k) * (hd ** -0.5), axis=-1)
    o = jnp.einsum('bhqk,bhkd->bhqd', A, v).transpose(0, 2, 1, 3).reshape(Bb, Ll, Dd)
    return o @ P['attn_out_w'].T + P['attn_out_b']


def _forward(f1, f2, f3, f4, params):
    Bb, Dd, h, w = f1.shape
    fs = [f.transpose(0, 2, 3, 1).reshape(Bb, h * w, Dd) for f in (f1, f2, f3, f4)]
    fs = _spatial_relation(fs, params)
    fs = [_multiscale(x, params) for x in fs]
    for lp in params['layers']:
        o = fs
        f1w2 = _cross_attn(o[0], o[1], lp['ca12']); f1w4 = _cross_attn(o[0], o[3], lp['ca14'])
        f2w1 = _cross_attn(o[1], o[0], lp['ca12']); f2w3 = _cross_attn(o[1], o[2], lp['ca23'])
        f3w2 = _cross_attn(o[2], o[1], lp['ca23']); f3w4 = _cross_attn(o[2], o[3], lp['ca34'])
        f4w1 = _cross_attn(o[3], o[0], lp['ca14']); f4w3 = _cross_attn(o[3], o[2], lp['ca34'])
        enh = [(f1w2 + f1w4) / 2, (f2w1 + f2w3) / 2, (f3w2 + f3w4) / 2, (f4w1 + f4w3) / 2]
        fs = [_layernorm(o[i] + jax.nn.sigmoid(_lin(enh[i], lp['gate'])) * enh[i],
                         lp['ln_g'], lp['ln_b'])
              for i in range(4)]
    pw = jax.nn.softmax(params['pool_w'])
    allf = jnp.concatenate([fs[i] * pw[i] for i in range(4)], -1)
    g = _layernorm(allf, params['gf_ln1_g'], params['gf_ln1_b'])
    g = _lin(_gelu(_lin(g, params['gf1'])), params['gf2'])
    g = _layernorm(g, params['gf_ln2_g'], params['gf_ln2_b'])
    attn = _mha(g, params)
    final = _layernorm(g + attn, params['sa_ln_g'], params['sa_ln_b'])
    return final.reshape(Bb, h, w, Dd).transpose(0, 3, 1, 2)


_pmapped = None


def _get_pmapped():
    global _pmapped
    if _pmapped is None:
        _pmapped = jax.pmap(_forward, in_axes=(0, 0, 0, 0, None))
    return _pmapped


def kernel(f1, f2, f3, f4, params):
    f1 = np.asarray(f1, dtype=np.float32)
    f2 = np.asarray(f2, dtype=np.float32)
    f3 = np.asarray(f3, dtype=np.float32)
    f4 = np.asarray(f4, dtype=np.float32)
    params = jax.tree_util.tree_map(lambda a: jnp.asarray(a, dtype=jnp.float32), params)

    sh = (N_CORES, B // N_CORES, D, HH, WW)
    fn = _get_pmapped()
    out = fn(f1.reshape(sh), f2.reshape(sh), f3.reshape(sh), f4.reshape(sh), params)
    out = np.asarray(out).reshape(B, D, HH, WW).astype(np.float32)
    return out
